# revision 9
# baseline (speedup 1.0000x reference)
"""Trainium2 Bass kernel for online sum-product belief propagation (PriorLayer).

Reference semantics (per step t): s_t = normalize((T @ s_{t-1}) * p_t)
Outputs: prior_probs (S, dim) = s_t for every t; uncertainty (S,) = entropy(s_t).

Parallelization: the normalized chain forgets its initial condition at ~3e-3
per step (positive random transition matrix => strong Birkhoff contraction),
so the sequence is split into independent chunks that each warm up for W
steps from a uniform init. W=8 puts the warm-up error ~1e-13, far below
fp32 noise. All chunks advance in lockstep as columns of a matmul.

Per-core layout (8 cores, data-parallel over 8192 positions each):
  - 256 chunks of length L=32, processed as 2 groups x 128 columns.
  - state u kept UNNORMALIZED in [dim(2x128 partitions), col] layout;
    a constant 1/alpha is folded into T so the scale stays O(1); each
    output column is normalized independently afterwards, which is exact.
  - per step: u = (Tn @ u) .* p_t  -> 4 matmuls [128x128x128] + 1 DVE mult.
  - output: PE transpose via identity-matmul augmented with a ones column
    (gives column sums for free), ACT normalizes + takes Ln from PSUM,
    DVE fused multiply-reduce produces the entropy dot product.
"""

import numpy as np

import concourse.bacc as bacc
import concourse.tile as tile
from concourse import mybir
from concourse.bass_utils import run_bass_kernel_spmd

F32 = mybir.dt.float32
AF = mybir.ActivationFunctionType
ALU = mybir.AluOpType

S_TOT = 65536
D = 256
N_CORES = 8
SD = S_TOT // N_CORES  # 8192 positions per core
W = 8                  # warm-up steps per chunk
L = 32                 # chunk length
G = 2                  # column groups of 128 chunks
C = 128                # columns (chunks) per group
KTOT = L + W           # lockstep steps per group
assert G * C * L == SD

_CACHE = {}


def _build(repeat=1):
    nc = bacc.Bacc("TRN2", target_bir_lowering=False, debug=False)

    pt = nc.dram_tensor("pt", [128, 2, KTOT, G * C], F32, kind="ExternalInput")
    tt = nc.dram_tensor("tt", [128, 2, D], F32, kind="ExternalInput")
    io = nc.dram_tensor("io", [128, 128], F32, kind="ExternalInput")
    m0 = nc.dram_tensor("m0", [128, 1], F32, kind="ExternalInput")
    m1 = nc.dram_tensor("m1", [128, 1], F32, kind="ExternalInput")
    prior = nc.dram_tensor("prior", [SD, D], F32, kind="ExternalOutput")
    unc = nc.dram_tensor("unc", [SD], F32, kind="ExternalOutput")

    prior_v = prior[:, :].rearrange("(c l) d -> c l d", l=L)   # [256, L, D]
    unc_v = unc[:].rearrange("(c l) -> c l", l=L)              # [256, L]

    with tile.TileContext(nc) as tc:
        with (
            tc.tile_pool(name="fix", bufs=1) as fix,
            tc.tile_pool(name="mm", bufs=3, space="PSUM") as mmp,
            tc.tile_pool(name="tr", bufs=3, space="PSUM") as trp,
            tc.tile_pool(name="yb", bufs=4) as yb,
            tc.tile_pool(name="lg", bufs=3) as lg,
            tc.tile_pool(name="sm", bufs=10) as sm,
        ):
            pt_sb = fix.tile([128, 2, KTOT, G * C], F32, tag="pt")
            tt_sb = fix.tile([128, 2, D], F32, tag="tt")
            io_sb = fix.tile([128, 128], F32, tag="io")
            m0_sb = fix.tile([128, 1], F32, tag="m0")
            eps_sb = fix.tile([128, 1], F32, tag="eps")
            m1_sb = fix.tile([128, 1], F32, tag="m1")
            u_sb = [fix.tile([128, 2, C], F32, tag=f"u{g}", name=f"u{g}") for g in range(G)]
            ent = [fix.tile([128, L], F32, tag=f"ent{g}", name=f"ent{g}") for g in range(G)]

            nc.sync.dma_start(out=tt_sb, in_=tt[:, :, :])
            nc.sync.dma_start(out=io_sb, in_=io[:, :])
            nc.sync.dma_start(out=m0_sb, in_=m0[:, :])
            nc.vector.memset(eps_sb, 1e-10)
            nc.sync.dma_start(out=m1_sb, in_=m1[:, :])

            for _rep in range(repeat):
                # stream p in k-chunks so compute starts early
                KC = 4
                for k0 in range(0, KTOT, KC):
                    k1 = min(k0 + KC, KTOT)
                    nc.sync.dma_start(
                        out=pt_sb[:, :, k0:k1, :],
                        in_=pt[:, :, k0:k1, :],
                    )

                for g in range(G):
                    nc.vector.memset(u_sb[g], 1.0 / D)

                for k in range(KTOT):
                    for g in range(G):
                        # u_new = Tn @ u  (contract over dim j in 2 halves)
                        mm = mmp.tile([128, 2, C], F32)
                        for h in range(2):
                            for kh in range(2):
                                nc.tensor.matmul(
                                    mm[:, h, :],
                                    tt_sb[:, kh, h * 128 : (h + 1) * 128],
                                    u_sb[g][:, kh, :],
                                    start=(kh == 0),
                                    stop=(kh == 1),
                                )
                        # u = u_new * p_t
                        nc.vector.tensor_tensor(
                            u_sb[g], mm, pt_sb[:, :, k, g * C : (g + 1) * C], ALU.mult
                        )
                        if k == W - 1 and g == 0:
                            # chunk 0 (core 0) starts exactly from uniform:
                            # u[:, :, 0] = u * m0 + m1  (m0=0, m1=1/D on core 0;
                            # identity elsewhere -- data-driven, SPMD-safe)
                            nc.gpsimd.tensor_scalar_mul(
                                u_sb[0][:, :, 0:1], u_sb[0][:, :, 0:1], m0_sb[:, 0:1]
                            )
                            nc.gpsimd.tensor_scalar_add(
                                u_sb[0][:, :, 0:1], u_sb[0][:, :, 0:1], m1_sb[:, 0:1]
                            )
                        if k >= W:
                            kk = k - W
                            # transpose u: tr[c, h, n] = u[h*128+n, c]
                            tr = trp.tile([128, 2, 128], F32)
                            for h in range(2):
                                nc.tensor.matmul(
                                    tr[:, h, :], u_sb[g][:, h, :], io_sb,
                                    start=True, stop=True,
                                )
                            # evacuate u^T to SBUF; column sums fall out of accum_out
                            ucp = yb.tile([128, 2, 128], F32, tag="ucp", bufs=4)
                            su = sm.tile([128, 1], F32, tag="su")
                            nc.scalar.activation(ucp, tr, AF.Copy, accum_out=su)
                            rsu = sm.tile([128, 1], F32, tag="rsu")
                            nc.vector.reciprocal(rsu, su)
                            logsu = sm.tile([128, 1], F32, tag="logsu")
                            nc.scalar.activation(logsu, su, AF.Ln, bias=eps_sb[:, 0:1])
                            # y = u^T * (1/sum)  -> normalized output row block
                            y = yb.tile([128, 2, 128], F32)
                            nc.gpsimd.tensor_scalar_mul(y, ucp, rsu[:, 0:1])
                            # logu = Ln(u + 1e-10)
                            logu = lg.tile([128, 2, 128], F32)
                            nc.scalar.activation(
                                logu, ucp, AF.Ln, bias=eps_sb[:, 0:1]
                            )
                            # sul = sum_j u * logu   (fused multiply-reduce;
                            # scalar_tensor_tensor is standard-ISA, unlike
                            # tensor_tensor_reduce whose custom-DVE table does
                            # not reach this runtime)
                            sul = sm.tile([128, 1], F32, tag="sul")
                            scr = lg.tile([128, 2, 128], F32, tag="scr")
                            nc.vector.scalar_tensor_tensor(
                                scr,
                                ucp,
                                1.0,
                                logu,
                                ALU.mult,
                                ALU.mult,
                                accum_out=sul,
                            )
                            # entropy = log(su) - sul/su
                            t1 = sm.tile([128, 1], F32, tag="t1")
                            nc.gpsimd.tensor_scalar_mul(t1, sul, rsu[:, 0:1])
                            nc.gpsimd.tensor_sub(ent[g][:, kk : kk + 1], logsu, t1)
                            nc.sync.dma_start(
                                out=prior_v[g * C : (g + 1) * C, kk, :], in_=y
                            )
                for g in range(G):
                    nc.sync.dma_start(out=unc_v[g * C : (g + 1) * C, :], in_=ent[g])

    nc.compile()
    return nc


def _host_prep(probs, transition_prior):
    probs = np.asarray(probs, dtype=np.float32)
    T = np.asarray(transition_prior, dtype=np.float32)

    alpha = np.float32(1.0 / (0.5 * T.sum(axis=1).mean()))
    tT = np.ascontiguousarray((T * alpha).T)           # [j, i]
    tt_h = tT.reshape(2, 128, D).transpose(1, 0, 2)    # [jp, kh, i]
    tt_h = np.ascontiguousarray(tt_h)

    io_h = np.eye(128, dtype=np.float32)

    pad = np.ones((W, D), np.float32)
    p_pad = np.concatenate([pad, probs], axis=0)       # [S+W, D]

    idx = (np.arange(G * C) * L)[None, :] + np.arange(KTOT)[:, None]  # [KTOT, 256]

    in_maps = []
    for d in range(N_CORES):
        arr = p_pad[d * SD : d * SD + SD + W]          # [SD+W, D]
        pt_d = arr[idx]                                # [KTOT, col, dim]
        pt_d = pt_d.transpose(2, 0, 1)                 # [dim, KTOT, col]
        pt_d = pt_d.reshape(2, 128, KTOT, G * C).transpose(1, 0, 2, 3)
        m0_h = np.full((128, 1), 0.0 if d == 0 else 1.0, np.float32)
        m1_h = np.full((128, 1), 1.0 / D if d == 0 else 0.0, np.float32)
        in_maps.append(
            {
                "pt": np.ascontiguousarray(pt_d),
                "tt": tt_h,
                "io": io_h,
                "m0": m0_h,
                "m1": m1_h,
            }
        )
    return in_maps


def _run(in_maps, repeat=1):
    if repeat not in _CACHE:
        _CACHE[repeat] = _build(repeat)
    nc = _CACHE[repeat]
    return run_bass_kernel_spmd(nc, in_maps, core_ids=list(range(N_CORES)))


def kernel(probs, transition_prior):
    in_maps = _host_prep(probs, transition_prior)
    res = _run(in_maps)
    prior = np.concatenate([r["prior"] for r in res.results], axis=0)
    uncert = np.concatenate([r["unc"] for r in res.results], axis=0)
    return prior, uncert


# revision 10
# speedup vs baseline: 1.5340x; 1.5340x over previous
"""Trainium2 Bass kernel for online sum-product belief propagation (PriorLayer).

Reference semantics (per step t): s_t = normalize((T @ s_{t-1}) * p_t)
Outputs: prior_probs (S, dim) = s_t for every t; uncertainty (S,) = entropy(s_t).

Parallelization: the normalized chain forgets its initial condition at ~3e-3
per step (positive random transition matrix => strong Birkhoff contraction),
so the sequence is split into independent chunks that each warm up for W
steps from a uniform init. W=8 puts the warm-up error ~1e-13, far below
fp32 noise. All chunks advance in lockstep as columns of a matmul.

Per-core layout (8 cores, data-parallel over 8192 positions each):
  - 256 chunks of length L=32, processed as 2 groups x 128 columns.
  - state u kept UNNORMALIZED in [dim(2x128 partitions), col] layout;
    a constant 1/alpha is folded into T so the scale stays O(1); each
    output column is normalized independently afterwards, which is exact.
  - per step: u = (Tn @ u) .* p_t  -> 4 matmuls [128x128x128] + 1 DVE mult.
  - output: PE transpose via identity-matmul augmented with a ones column
    (gives column sums for free), ACT normalizes + takes Ln from PSUM,
    DVE fused multiply-reduce produces the entropy dot product.
"""

import numpy as np

import concourse.bacc as bacc
import concourse.tile as tile
from concourse import mybir
from concourse.bass_utils import run_bass_kernel_spmd

F32 = mybir.dt.float32
AF = mybir.ActivationFunctionType
ALU = mybir.AluOpType

S_TOT = 65536
D = 256
N_CORES = 8
SD = S_TOT // N_CORES  # 8192 positions per core
W = 8                  # warm-up steps per chunk
L = 32                 # chunk length
G = 2                  # column groups of 128 chunks
C = 128                # columns (chunks) per group
KTOT = L + W           # lockstep steps per group
assert G * C * L == SD

_CACHE = {}


def _build(repeat=1):
    nc = bacc.Bacc("TRN2", target_bir_lowering=False, debug=False)

    pt = nc.dram_tensor("pt", [128, 2, KTOT, G * C], F32, kind="ExternalInput")
    tt = nc.dram_tensor("tt", [128, 2, D], F32, kind="ExternalInput")
    io = nc.dram_tensor("io", [128, 129], F32, kind="ExternalInput")
    m0 = nc.dram_tensor("m0", [128, 1], F32, kind="ExternalInput")
    m1 = nc.dram_tensor("m1", [128, 1], F32, kind="ExternalInput")
    prior = nc.dram_tensor("prior", [SD, D], F32, kind="ExternalOutput")
    unc = nc.dram_tensor("unc", [SD], F32, kind="ExternalOutput")

    prior_v = prior[:, :].rearrange("(c l) d -> c l d", l=L)   # [256, L, D]
    unc_v = unc[:].rearrange("(c l) -> c l", l=L)              # [256, L]

    with tile.TileContext(nc) as tc:
        with (
            tc.tile_pool(name="fix", bufs=1) as fix,
            tc.tile_pool(name="mm", bufs=3, space="PSUM") as mmp,
            tc.tile_pool(name="tr", bufs=3, space="PSUM") as trp,
            tc.tile_pool(name="yb", bufs=4) as yb,
            tc.tile_pool(name="lg", bufs=3) as lg,
            tc.tile_pool(name="sm", bufs=10) as sm,
        ):
            pt_sb = fix.tile([128, 2, KTOT, G * C], F32, tag="pt")
            tt_sb = fix.tile([128, 2, D], F32, tag="tt")
            io_sb = fix.tile([128, 129], F32, tag="io")
            m0_sb = fix.tile([128, 1], F32, tag="m0")
            eps_sb = fix.tile([128, 1], F32, tag="eps")
            m1_sb = fix.tile([128, 1], F32, tag="m1")
            u_sb = [fix.tile([128, 2, C], F32, tag=f"u{g}", name=f"u{g}") for g in range(G)]
            ent = [fix.tile([128, L], F32, tag=f"ent{g}", name=f"ent{g}") for g in range(G)]

            nc.sync.dma_start(out=tt_sb, in_=tt[:, :, :])
            nc.sync.dma_start(out=io_sb, in_=io[:, :])
            nc.sync.dma_start(out=m0_sb, in_=m0[:, :])
            nc.vector.memset(eps_sb, 1e-10)
            nc.sync.dma_start(out=m1_sb, in_=m1[:, :])

            for _rep in range(repeat):
                # stream p in k-chunks so compute starts early
                KC = 4
                for k0 in range(0, KTOT, KC):
                    k1 = min(k0 + KC, KTOT)
                    nc.sync.dma_start(
                        out=pt_sb[:, :, k0:k1, :],
                        in_=pt[:, :, k0:k1, :],
                    )

                for g in range(G):
                    nc.vector.memset(u_sb[g], 1.0 / D)

                for k in range(KTOT):
                    for g in range(G):
                        # u_new = Tn @ u  (contract over dim j in 2 halves)
                        mm = mmp.tile([128, 2, C], F32)
                        for h in range(2):
                            for kh in range(2):
                                nc.tensor.matmul(
                                    mm[:, h, :],
                                    tt_sb[:, kh, h * 128 : (h + 1) * 128],
                                    u_sb[g][:, kh, :],
                                    start=(kh == 0),
                                    stop=(kh == 1),
                                )
                        # u = u_new * p_t
                        nc.vector.tensor_tensor(
                            u_sb[g], mm, pt_sb[:, :, k, g * C : (g + 1) * C], ALU.mult
                        )
                        if k == W - 1 and g == 0:
                            # chunk 0 (core 0) starts exactly from uniform:
                            # u[:, :, 0] = u * m0 + m1  (m0=0, m1=1/D on core 0;
                            # identity elsewhere -- data-driven, SPMD-safe)
                            nc.vector.tensor_scalar(
                                u_sb[0][:, :, 0:1],
                                u_sb[0][:, :, 0:1],
                                m0_sb[:, 0:1],
                                m1_sb[:, 0:1],
                                ALU.mult,
                                ALU.add,
                            )
                        if k >= W:
                            kk = k - W
                            # transpose u + column sums via [I | 1] rhs:
                            # tr[c, h, n<128] = u[h*128+n, c]; tr[c, 0, 128] = sum_j u[j, c]
                            tr = trp.tile([128, 2, 129], F32)
                            nc.tensor.matmul(
                                tr[:, 0, :], u_sb[g][:, 0, :], io_sb,
                                start=True, stop=False,
                            )
                            nc.tensor.matmul(
                                tr[:, 0, 128:129], u_sb[g][:, 1, :],
                                io_sb[:, 128:129],
                                start=False, stop=True,
                            )
                            nc.tensor.matmul(
                                tr[:, 1, :], u_sb[g][:, 1, :], io_sb,
                                start=True, stop=True,
                            )
                            su = tr[:, 0, 128:129]
                            rsu = sm.tile([128, 1], F32, tag="rsu")
                            nc.vector.reciprocal(rsu, su)
                            logsu = sm.tile([128, 1], F32, tag="logsu")
                            nc.scalar.activation(logsu, su, AF.Ln, bias=eps_sb[:, 0:1])
                            # y = u^T * (1/sum)  -> normalized output row block
                            y = yb.tile([128, 2, 128], F32)
                            nc.scalar.activation(y, tr[:, :, 0:128], AF.Copy, scale=rsu)
                            # logu = Ln(u + 1e-10)
                            logu = lg.tile([128, 2, 128], F32)
                            nc.scalar.activation(
                                logu, tr[:, :, 0:128], AF.Ln, bias=eps_sb[:, 0:1]
                            )
                            # sul = sum_j u * logu (fused multiply-reduce, standard ISA)
                            sul = sm.tile([128, 1], F32, tag="sul")
                            scr = lg.tile([128, 2, 128], F32, tag="scr")
                            nc.vector.scalar_tensor_tensor(
                                scr,
                                tr[:, :, 0:128],
                                1.0,
                                logu,
                                ALU.mult,
                                ALU.mult,
                                accum_out=sul,
                            )
                            # NEGATED entropy: (sul*rsu) - logsu; sign fixed in bulk below
                            nc.vector.tensor_scalar(
                                ent[g][:, kk : kk + 1],
                                sul,
                                rsu[:, 0:1],
                                logsu[:, 0:1],
                                ALU.mult,
                                ALU.subtract,
                            )
                            nc.sync.dma_start(
                                out=prior_v[g * C : (g + 1) * C, kk, :], in_=y
                            )
                for g in range(G):
                    nc.vector.tensor_scalar_mul(ent[g], ent[g], -1.0)
                    nc.sync.dma_start(out=unc_v[g * C : (g + 1) * C, :], in_=ent[g])

    nc.compile()
    return nc


def _host_prep(probs, transition_prior):
    probs = np.asarray(probs, dtype=np.float32)
    T = np.asarray(transition_prior, dtype=np.float32)

    alpha = np.float32(1.0 / (0.5 * T.sum(axis=1).mean()))
    tT = np.ascontiguousarray((T * alpha).T)           # [j, i]
    tt_h = tT.reshape(2, 128, D).transpose(1, 0, 2)    # [jp, kh, i]
    tt_h = np.ascontiguousarray(tt_h)

    io_h = np.zeros((128, 129), np.float32)
    io_h[:, :128] = np.eye(128, dtype=np.float32)
    io_h[:, 128] = 1.0

    pad = np.ones((W, D), np.float32)
    p_pad = np.concatenate([pad, probs], axis=0)       # [S+W, D]

    idx = (np.arange(G * C) * L)[None, :] + np.arange(KTOT)[:, None]  # [KTOT, 256]

    in_maps = []
    for d in range(N_CORES):
        arr = p_pad[d * SD : d * SD + SD + W]          # [SD+W, D]
        pt_d = arr[idx]                                # [KTOT, col, dim]
        pt_d = pt_d.transpose(2, 0, 1)                 # [dim, KTOT, col]
        pt_d = pt_d.reshape(2, 128, KTOT, G * C).transpose(1, 0, 2, 3)
        m0_h = np.full((128, 1), 0.0 if d == 0 else 1.0, np.float32)
        m1_h = np.full((128, 1), 1.0 / D if d == 0 else 0.0, np.float32)
        in_maps.append(
            {
                "pt": np.ascontiguousarray(pt_d),
                "tt": tt_h,
                "io": io_h,
                "m0": m0_h,
                "m1": m1_h,
            }
        )
    return in_maps


def _run(in_maps, repeat=1):
    if repeat not in _CACHE:
        _CACHE[repeat] = _build(repeat)
    nc = _CACHE[repeat]
    return run_bass_kernel_spmd(nc, in_maps, core_ids=list(range(N_CORES)))


def kernel(probs, transition_prior):
    in_maps = _host_prep(probs, transition_prior)
    res = _run(in_maps)
    prior = np.concatenate([r["prior"] for r in res.results], axis=0)
    uncert = np.concatenate([r["unc"] for r in res.results], axis=0)
    return prior, uncert
